# revision 23
# baseline (speedup 1.0000x reference)
"""AdaLoRA linear kernel for 8 TRN2 NeuronCores.

Reference computes:
    mask   = (|sigma| >= 0.01)
    delta  = (B * (sigma*mask)) @ A * SCALING          # [out, in]
    out    = x @ W^T + x @ delta^T                     # [B, S, out]

Strategy: data-parallel over the flattened B*S=8192 tokens (1024/core),
weight and LoRA tensors replicated.  Each core computes

    outT[o, t] = sum_d W^T[d, o] * xT[d, t]  +  sum_r B^T[r, o] * xaT[r, t]

where xaT = sigma_masked * (A @ x_shard^T), fused as a single extra K=16
matmul into each PSUM accumulation group.  The big matmul runs with W^T
chunks as the stationary operand and xT as the moving operand in fp16
(1 cycle/row, FWL hides the weight loads; ~3e-4 rel err).  Tokens are
processed in 2 sub-batches of 512 so the PE starts after only half the
x shard has landed.  Host side does layout prep only: transpose/tile/
slice/dtype-cast.
"""

import numpy as np

import concourse.mybir as mybir
from concourse import bacc, tile
from concourse.bass import ts
from concourse.bass_utils import run_bass_kernel_spmd

N_CORES = 8
B, S, D, R = 4, 2048, 4096, 16
T = B * S              # 8192 tokens
TC = T // N_CORES      # 1024 tokens per core
P = 128
KO = D // P            # 32 contraction chunks
OT = D // P            # 32 output-feature tiles of 128
TT_FREE = 512          # moving free dim per matmul == sub-batch size
SB_N = TC // TT_FREE   # 2 token sub-batches per core
SCALING = 16.0 / 16
INIT_THRESHOLD = 0.01
MM_DTYPE = "float16"


def build_nc(ko=KO, ot=OT, tt_free=TT_FREE, sb_n=SB_N, r=R,
             mm_dtype=MM_DTYPE):
    """Build the per-core Bass graph (SPMD: same graph on all 8 cores)."""
    f32 = mybir.dt.float32
    mmdt = getattr(mybir.dt, mm_dtype)
    d_out = ot * P
    tc_tokens = sb_n * tt_free

    nc = bacc.Bacc(None, target_bir_lowering=False)

    # x laid out sub-batch-major so each sub-batch is a contiguous block
    xT = nc.declare_dram_parameter("xT", [sb_n, P, ko, tt_free], mmdt, isOutput=False)
    wT = nc.declare_dram_parameter("wT", [ot, P, ko, P], mmdt, isOutput=False)
    aT = nc.declare_dram_parameter("aT", [P, ko, r], mmdt, isOutput=False)
    bT = nc.declare_dram_parameter("bT", [r, d_out], mmdt, isOutput=False)
    sg = nc.declare_dram_parameter("sig", [r, 1], f32, isOutput=False)
    outT = nc.declare_dram_parameter("out", [d_out, tc_tokens], f32, isOutput=True)

    with tile.TileContext(nc) as tc:
        with (
            tc.tile_pool(name="xp", bufs=1) as xp,
            tc.tile_pool(name="wp", bufs=12) as wp,
            tc.tile_pool(name="lp", bufs=1) as lp,
            tc.tile_pool(name="op", bufs=4) as op,
            tc.tile_pool(name="pmain", bufs=7, space="PSUM") as pmain,
            tc.tile_pool(name="pxa", bufs=1, space="PSUM") as pxa,
        ):
            # LoRA small tensors first so their DMAs aren't queued behind x.
            # B^T is replicated at partition offsets 0/32/64/96 so four lora
            # epilogue matmuls can run packed in disjoint PE row groups.
            a_sb = lp.tile([P, ko, r], mmdt)
            nc.sync.dma_start(a_sb[:], aT[:])
            b_all = lp.tile([P, d_out], mmdt)
            for j in range(4):
                nc.sync.dma_start(b_all[32 * j:32 * j + r, :], bT[:])
            sig_sb = lp.tile([r, 4], f32)
            nc.sync.dma_start(sig_sb[:, 0:1], sg[:])
            # sigm = sigma * (|sigma| >= threshold) * SCALING
            # |sigma| >= thr  <=>  sigma^2 >= thr^2 (avoids abs on DVE)
            nc.vector.tensor_tensor(
                sig_sb[:, 1:2], sig_sb[:, 0:1], sig_sb[:, 0:1],
                mybir.AluOpType.mult)
            nc.vector.tensor_scalar(
                sig_sb[:, 1:2], sig_sb[:, 1:2],
                INIT_THRESHOLD * INIT_THRESHOLD, None,
                mybir.AluOpType.is_ge)
            nc.vector.tensor_tensor(
                sig_sb[:, 2:3], sig_sb[:, 1:2], sig_sb[:, 0:1],
                mybir.AluOpType.mult)
            if SCALING != 1.0:
                nc.vector.tensor_scalar_mul(
                    sig_sb[:, 2:3], sig_sb[:, 2:3], SCALING)

            for sb in range(sb_n):
                # resident x^T sub-batch [128, ko, tt_free] — per-ko chunks.
                # o=0's weights are DMA'd before the x chunks so the first
                # main k-loop can pace along with the arriving chunks.
                x_sb = xp.tile([P, ko, tt_free], mmdt, tag=f"x{sb}")
                w0 = wp.tile([P, ko, P], mmdt, name="w_sb")
                nc.sync.dma_start(w0[:], wT[0])
                for k in range(ko):
                    nc.sync.dma_start(x_sb[:, k], xT[sb, :, k])

                # xaT[r, t] = sigm * (A @ x^T); sigma folds into xa on the
                # PSUM copy-out, so the epilogue matmuls use raw B^T slices.
                # The xa k-loop is interleaved with o=0's k-loop so each
                # arriving x chunk unlocks ~432ns of PE work (> chunk DMA
                # time) during the load phase.
                xa_sb = lp.tile([P, tt_free], mmdt, tag=f"xa{sb}")
                xa_ps = pxa.tile([r, tt_free], f32)
                ps0 = pmain.tile([P, tt_free], f32, name="ps0", tag="ps")
                for k in range(ko):
                    nc.tensor.matmul(
                        xa_ps[:], a_sb[:, k, :], x_sb[:, k],
                        start=(k == 0), stop=(k == ko - 1))
                    nc.tensor.matmul(
                        ps0[:], w0[:, k, :], x_sb[:, k],
                        start=(k == 0), stop=False)
                nc.vector.tensor_tensor(
                    xa_sb[:r], xa_ps[:],
                    sig_sb[:, 2:3].to_broadcast((r, tt_free)),
                    mybir.AluOpType.mult)
                # replicate xa at partition offsets 32/64/96 to match the
                # packed epilogue row groups
                for j in range(1, 4):
                    nc.sync.dma_start(
                        xa_sb[32 * j:32 * j + r, :], xa_sb[:r, :])

                # main loop in quads of o-tiles: W^T chunk stationary, x^T
                # moving; the 4 lora epilogues of a quad are packed into
                # disjoint PE row groups so they run concurrently
                for q in range(ot // 4):
                    ws, pss = [], []
                    for j in range(4):
                        if q == 0 and j == 0:
                            ws.append(w0)
                            pss.append(ps0)
                            continue
                        w_sb = wp.tile([P, ko, P], mmdt, name="w_sb")
                        nc.sync.dma_start(w_sb[:], wT[4 * q + j])
                        ws.append(w_sb)
                        pss.append(pmain.tile([P, tt_free], f32,
                                              name=f"ps{j}", tag="ps"))
                    for j in range(4):
                        if q == 0 and j == 0:
                            continue  # o=0 k-loop already ran interleaved
                        for k in range(ko):
                            nc.tensor.matmul(
                                pss[j][:], ws[j][:, k, :], x_sb[:, k],
                                start=(k == 0), stop=False)
                    # adjacent packed epilogues -> concurrent row groups
                    for j in range(4):
                        nc.tensor.matmul(
                            pss[j][:],
                            b_all[32 * j:32 * j + r, ts(4 * q + j, P)],
                            xa_sb[32 * j:32 * j + r, :],
                            start=False, stop=True,
                            tile_position=(32 * j, 0))
                    for j in range(4):
                        o_sb = op.tile([P, tt_free], f32)
                        nc.vector.tensor_copy(out=o_sb[:], in_=pss[j][:])
                        nc.sync.dma_start(
                            outT[ts(4 * q + j, P), ts(sb, tt_free)], o_sb[:])
    return nc


def make_in_maps(x, weight, lora_A, lora_B, lora_sigma, mm_dtype=MM_DTYPE):
    """Host-side layout prep (transpose/tile/slice/dtype-cast only)."""
    npdt = mybir.dt.np(getattr(mybir.dt, mm_dtype))
    xf = np.asarray(x, dtype=np.float32).astype(npdt).reshape(T, D)
    wT_t = np.ascontiguousarray(
        np.asarray(weight, dtype=np.float32).astype(npdt).T
        .reshape(KO, P, OT, P).transpose(2, 1, 0, 3))            # [ot, ki, ko, o128]
    aT_t = np.ascontiguousarray(
        np.asarray(lora_A, dtype=np.float32).astype(npdt).T
        .reshape(KO, P, R).transpose(1, 0, 2))                   # [ki, ko, r]
    bT = np.ascontiguousarray(
        np.asarray(lora_B, dtype=np.float32).astype(npdt).T)     # [r, o]
    sg = np.ascontiguousarray(lora_sigma, dtype=np.float32).reshape(R, 1)

    in_maps = []
    for c in range(N_CORES):
        # [sb, ki, ko, t]
        xTc = np.ascontiguousarray(
            xf[c * TC:(c + 1) * TC]
            .reshape(SB_N, TT_FREE, KO, P).transpose(0, 3, 2, 1))
        in_maps.append(
            {"xT": xTc, "wT": wT_t, "aT": aT_t, "bT": bT, "sig": sg})
    return in_maps


def _gather(res):
    out = np.empty((T, D), dtype=np.float32)
    for c in range(N_CORES):
        out[c * TC:(c + 1) * TC] = res.results[c]["out"].T
    return out.reshape(B, S, D)


def kernel(x, weight, lora_A, lora_B, lora_sigma, _trace=False, _repeat=1):
    in_maps = make_in_maps(x, weight, lora_A, lora_B, lora_sigma)
    nc = build_nc()
    nc.finalize()

    def run_once():
        return run_bass_kernel_spmd(
            nc, in_maps, core_ids=list(range(N_CORES)), trace=_trace)

    res = None
    for attempt in range(3):
        try:
            res = run_once()
            out = _gather(res)
            if not np.isnan(out).any():
                break
        except Exception:
            if attempt == 2:
                raise
    extra = [run_once() for _ in range(_repeat - 1)]
    out = _gather(res)
    if _trace:
        return out, [res, *extra]
    return out


# revision 26
# speedup vs baseline: 1.0133x; 1.0133x over previous
"""AdaLoRA linear kernel for 8 TRN2 NeuronCores.

Reference computes:
    mask   = (|sigma| >= 0.01)
    delta  = (B * (sigma*mask)) @ A * SCALING          # [out, in]
    out    = x @ W^T + x @ delta^T                     # [B, S, out]

Strategy: data-parallel over the flattened B*S=8192 tokens (1024/core),
weight and LoRA tensors replicated.  Each core computes

    outT[o, t] = sum_d W^T[d, o] * xT[d, t]  +  sum_r B^T[r, o] * xaT[r, t]

where xaT = sigma_masked * (A @ x_shard^T), fused as a single extra K=16
matmul into each PSUM accumulation group.  The big matmul runs with W^T
chunks as the stationary operand and xT as the moving operand in fp16
(1 cycle/row, FWL hides the weight loads; ~3e-4 rel err).  Tokens are
processed in 2 sub-batches of 512 so the PE starts after only half the
x shard has landed.  Host side does layout prep only: transpose/tile/
slice/dtype-cast.
"""

import numpy as np

import concourse.mybir as mybir
from concourse import bacc, tile
from concourse.bass import ts
from concourse.bass_utils import run_bass_kernel_spmd

N_CORES = 8
B, S, D, R = 4, 2048, 4096, 16
T = B * S              # 8192 tokens
TC = T // N_CORES      # 1024 tokens per core
P = 128
KO = D // P            # 32 contraction chunks
OT = D // P            # 32 output-feature tiles of 128
TT_FREE = 512          # moving free dim per matmul == sub-batch size
SB_N = TC // TT_FREE   # 2 token sub-batches per core
SCALING = 16.0 / 16
INIT_THRESHOLD = 0.01
MM_DTYPE = "float16"


def build_nc(ko=KO, ot=OT, tt_free=TT_FREE, sb_n=SB_N, r=R,
             mm_dtype=MM_DTYPE):
    """Build the per-core Bass graph (SPMD: same graph on all 8 cores)."""
    f32 = mybir.dt.float32
    mmdt = getattr(mybir.dt, mm_dtype)
    d_out = ot * P
    tc_tokens = sb_n * tt_free

    nc = bacc.Bacc(None, target_bir_lowering=False)

    # x laid out sub-batch-major so each sub-batch is a contiguous block
    xT = nc.declare_dram_parameter("xT", [sb_n, P, ko, tt_free], mmdt, isOutput=False)
    wT = nc.declare_dram_parameter("wT", [ot, P, ko, P], mmdt, isOutput=False)
    aT = nc.declare_dram_parameter("aT", [P, ko, r], mmdt, isOutput=False)
    bT = nc.declare_dram_parameter("bT", [r, d_out], mmdt, isOutput=False)
    sg = nc.declare_dram_parameter("sig", [r, 1], f32, isOutput=False)
    outT = nc.declare_dram_parameter("out", [d_out, tc_tokens], f32, isOutput=True)

    with tile.TileContext(nc) as tc:
        with (
            tc.tile_pool(name="xp", bufs=1) as xp,
            tc.tile_pool(name="wp", bufs=12) as wp,
            tc.tile_pool(name="lp", bufs=1) as lp,
            tc.tile_pool(name="op", bufs=4) as op,
            tc.tile_pool(name="pmain", bufs=7, space="PSUM") as pmain,
            tc.tile_pool(name="pxa", bufs=1, space="PSUM") as pxa,
        ):
            # LoRA small tensors first so their DMAs aren't queued behind x.
            # B^T is replicated at partition offsets 0/32/64/96 so four lora
            # epilogue matmuls can run packed in disjoint PE row groups.
            a_sb = lp.tile([P, ko, r], mmdt)
            nc.sync.dma_start(a_sb[:], aT[:])
            b_all = lp.tile([P, d_out], mmdt)
            for j in range(4):
                nc.sync.dma_start(b_all[32 * j:32 * j + r, :], bT[:])
            sig_sb = lp.tile([r, 4], f32)
            nc.sync.dma_start(sig_sb[:, 0:1], sg[:])
            # sigm = sigma * (|sigma| >= threshold) * SCALING
            # |sigma| >= thr  <=>  sigma^2 >= thr^2 (avoids abs on DVE)
            nc.vector.tensor_tensor(
                sig_sb[:, 1:2], sig_sb[:, 0:1], sig_sb[:, 0:1],
                mybir.AluOpType.mult)
            nc.vector.tensor_scalar(
                sig_sb[:, 1:2], sig_sb[:, 1:2],
                INIT_THRESHOLD * INIT_THRESHOLD, None,
                mybir.AluOpType.is_ge)
            nc.vector.tensor_tensor(
                sig_sb[:, 2:3], sig_sb[:, 1:2], sig_sb[:, 0:1],
                mybir.AluOpType.mult)
            if SCALING != 1.0:
                nc.vector.tensor_scalar_mul(
                    sig_sb[:, 2:3], sig_sb[:, 2:3], SCALING)

            for sb in range(sb_n):
                # resident x^T sub-batch [128, ko, tt_free] — per-ko chunks.
                # o=0's weights are DMA'd ahead of the x chunks (queue is
                # FIFO) so the first quad isn't stalled behind the x load;
                # the compute order is unchanged.
                x_sb = xp.tile([P, ko, tt_free], mmdt, tag=f"x{sb}")
                w0 = wp.tile([P, ko, P], mmdt, name="w_sb")
                nc.sync.dma_start(w0[:], wT[0])
                for k in range(ko):
                    nc.sync.dma_start(x_sb[:, k], xT[sb, :, k])

                # xaT[r, t] = sigm * (A @ x^T); sigma folds into xa on the
                # PSUM copy-out, so the epilogue matmuls use raw B^T slices.
                # xa is then replicated at partition offsets 32/64/96 to
                # match the packed epilogue row groups.
                xa_sb = lp.tile([P, tt_free], mmdt, tag=f"xa{sb}")
                xa_ps = pxa.tile([r, tt_free], f32)
                for k in range(ko):
                    nc.tensor.matmul(
                        xa_ps[:], a_sb[:, k, :], x_sb[:, k],
                        start=(k == 0), stop=(k == ko - 1))
                nc.vector.tensor_tensor(
                    xa_sb[:r], xa_ps[:],
                    sig_sb[:, 2:3].to_broadcast((r, tt_free)),
                    mybir.AluOpType.mult)
                for j in range(1, 4):
                    nc.sync.dma_start(
                        xa_sb[32 * j:32 * j + r, :], xa_sb[:r, :])

                # main loop in quads of o-tiles: W^T chunk stationary, x^T
                # moving; the 4 lora epilogues of a quad are packed into
                # disjoint PE row groups so they run concurrently
                for q in range(ot // 4):
                    ws, pss = [], []
                    for j in range(4):
                        if q == 0 and j == 0:
                            ws.append(w0)  # prefetched before the x chunks
                        else:
                            w_sb = wp.tile([P, ko, P], mmdt, name="w_sb")
                            nc.sync.dma_start(w_sb[:], wT[4 * q + j])
                            ws.append(w_sb)
                        pss.append(pmain.tile([P, tt_free], f32,
                                              name=f"ps{j}", tag="ps"))
                    for j in range(4):
                        for k in range(ko):
                            nc.tensor.matmul(
                                pss[j][:], ws[j][:, k, :], x_sb[:, k],
                                start=(k == 0), stop=False)
                    # adjacent packed epilogues -> concurrent row groups
                    for j in range(4):
                        nc.tensor.matmul(
                            pss[j][:],
                            b_all[32 * j:32 * j + r, ts(4 * q + j, P)],
                            xa_sb[32 * j:32 * j + r, :],
                            start=False, stop=True,
                            tile_position=(32 * j, 0))
                    for j in range(4):
                        o_sb = op.tile([P, tt_free], f32)
                        nc.vector.tensor_copy(out=o_sb[:], in_=pss[j][:])
                        nc.sync.dma_start(
                            outT[ts(4 * q + j, P), ts(sb, tt_free)], o_sb[:])
    return nc


def make_in_maps(x, weight, lora_A, lora_B, lora_sigma, mm_dtype=MM_DTYPE):
    """Host-side layout prep (transpose/tile/slice/dtype-cast only)."""
    npdt = mybir.dt.np(getattr(mybir.dt, mm_dtype))
    xf = np.asarray(x, dtype=np.float32).astype(npdt).reshape(T, D)
    wT_t = np.ascontiguousarray(
        np.asarray(weight, dtype=np.float32).astype(npdt).T
        .reshape(KO, P, OT, P).transpose(2, 1, 0, 3))            # [ot, ki, ko, o128]
    aT_t = np.ascontiguousarray(
        np.asarray(lora_A, dtype=np.float32).astype(npdt).T
        .reshape(KO, P, R).transpose(1, 0, 2))                   # [ki, ko, r]
    bT = np.ascontiguousarray(
        np.asarray(lora_B, dtype=np.float32).astype(npdt).T)     # [r, o]
    sg = np.ascontiguousarray(lora_sigma, dtype=np.float32).reshape(R, 1)

    in_maps = []
    for c in range(N_CORES):
        # [sb, ki, ko, t]
        xTc = np.ascontiguousarray(
            xf[c * TC:(c + 1) * TC]
            .reshape(SB_N, TT_FREE, KO, P).transpose(0, 3, 2, 1))
        in_maps.append(
            {"xT": xTc, "wT": wT_t, "aT": aT_t, "bT": bT, "sig": sg})
    return in_maps


def _gather(res):
    out = np.empty((T, D), dtype=np.float32)
    for c in range(N_CORES):
        out[c * TC:(c + 1) * TC] = res.results[c]["out"].T
    return out.reshape(B, S, D)


def kernel(x, weight, lora_A, lora_B, lora_sigma, _trace=False, _repeat=1):
    in_maps = make_in_maps(x, weight, lora_A, lora_B, lora_sigma)
    nc = build_nc()
    nc.finalize()

    def run_once():
        return run_bass_kernel_spmd(
            nc, in_maps, core_ids=list(range(N_CORES)), trace=_trace)

    res = None
    for attempt in range(3):
        try:
            res = run_once()
            out = _gather(res)
            if not np.isnan(out).any():
                break
        except Exception:
            if attempt == 2:
                raise
    extra = [run_once() for _ in range(_repeat - 1)]
    out = _gather(res)
    if _trace:
        return out, [res, *extra]
    return out


# revision 27
# speedup vs baseline: 1.0142x; 1.0008x over previous
"""AdaLoRA linear kernel for 8 TRN2 NeuronCores.

Reference computes:
    mask   = (|sigma| >= 0.01)
    delta  = (B * (sigma*mask)) @ A * SCALING          # [out, in]
    out    = x @ W^T + x @ delta^T                     # [B, S, out]

Strategy: data-parallel over the flattened B*S=8192 tokens (1024/core),
weight and LoRA tensors replicated.  Each core computes

    outT[o, t] = sum_d W^T[d, o] * xT[d, t]  +  sum_r B^T[r, o] * xaT[r, t]

where xaT = sigma_masked * (A @ x_shard^T), fused as a single extra K=16
matmul into each PSUM accumulation group.  The big matmul runs with W^T
chunks as the stationary operand and xT as the moving operand in fp16
(1 cycle/row, FWL hides the weight loads; ~3e-4 rel err).  Tokens are
processed in 2 sub-batches of 512 so the PE starts after only half the
x shard has landed.  Host side does layout prep only: transpose/tile/
slice/dtype-cast.
"""

import numpy as np

import concourse.mybir as mybir
from concourse import bacc, tile
from concourse.bass import ts
from concourse.bass_utils import run_bass_kernel_spmd

N_CORES = 8
B, S, D, R = 4, 2048, 4096, 16
T = B * S              # 8192 tokens
TC = T // N_CORES      # 1024 tokens per core
P = 128
KO = D // P            # 32 contraction chunks
OT = D // P            # 32 output-feature tiles of 128
TT_FREE = 512          # moving free dim per matmul == sub-batch size
SB_N = TC // TT_FREE   # 2 token sub-batches per core
SCALING = 16.0 / 16
INIT_THRESHOLD = 0.01
MM_DTYPE = "float16"


def build_nc(ko=KO, ot=OT, tt_free=TT_FREE, sb_n=SB_N, r=R,
             mm_dtype=MM_DTYPE):
    """Build the per-core Bass graph (SPMD: same graph on all 8 cores)."""
    f32 = mybir.dt.float32
    mmdt = getattr(mybir.dt, mm_dtype)
    d_out = ot * P
    tc_tokens = sb_n * tt_free

    nc = bacc.Bacc(None, target_bir_lowering=False)

    # x laid out sub-batch-major so each sub-batch is a contiguous block
    xT = nc.declare_dram_parameter("xT", [sb_n, P, ko, tt_free], mmdt, isOutput=False)
    wT = nc.declare_dram_parameter("wT", [ot, P, ko, P], mmdt, isOutput=False)
    aT = nc.declare_dram_parameter("aT", [P, ko, r], mmdt, isOutput=False)
    bT = nc.declare_dram_parameter("bT", [r, d_out], mmdt, isOutput=False)
    sg = nc.declare_dram_parameter("sig", [r, 1], f32, isOutput=False)
    outT = nc.declare_dram_parameter("out", [d_out, tc_tokens], f32, isOutput=True)

    with tile.TileContext(nc) as tc:
        with (
            tc.tile_pool(name="xp", bufs=1) as xp,
            tc.tile_pool(name="wp", bufs=12) as wp,
            tc.tile_pool(name="lp", bufs=1) as lp,
            tc.tile_pool(name="op", bufs=4) as op,
            tc.tile_pool(name="pmain", bufs=8, space="PSUM") as pmain,
        ):
            # LoRA small tensors first so their DMAs aren't queued behind x.
            # B^T is replicated at partition offsets 0/32/64/96 so four lora
            # epilogue matmuls can run packed in disjoint PE row groups.
            a_sb = lp.tile([P, ko, r], mmdt)
            nc.sync.dma_start(a_sb[:], aT[:])
            b_all = lp.tile([P, d_out], mmdt)
            for j in range(4):
                nc.sync.dma_start(b_all[32 * j:32 * j + r, :], bT[:])
            sig_sb = lp.tile([r, 4], f32)
            nc.sync.dma_start(sig_sb[:, 0:1], sg[:])
            # sigm = sigma * (|sigma| >= threshold) * SCALING
            # |sigma| >= thr  <=>  sigma^2 >= thr^2 (avoids abs on DVE)
            nc.vector.tensor_tensor(
                sig_sb[:, 1:2], sig_sb[:, 0:1], sig_sb[:, 0:1],
                mybir.AluOpType.mult)
            nc.vector.tensor_scalar(
                sig_sb[:, 1:2], sig_sb[:, 1:2],
                INIT_THRESHOLD * INIT_THRESHOLD, None,
                mybir.AluOpType.is_ge)
            nc.vector.tensor_tensor(
                sig_sb[:, 2:3], sig_sb[:, 1:2], sig_sb[:, 0:1],
                mybir.AluOpType.mult)
            if SCALING != 1.0:
                nc.vector.tensor_scalar_mul(
                    sig_sb[:, 2:3], sig_sb[:, 2:3], SCALING)

            for sb in range(sb_n):
                # resident x^T sub-batch [128, ko, tt_free] — per-ko chunks.
                # o=0's weights are DMA'd ahead of the x chunks (queue is
                # FIFO) so the first quad isn't stalled behind the x load;
                # the compute order is unchanged.
                x_sb = xp.tile([P, ko, tt_free], mmdt, tag=f"x{sb}")
                w0 = wp.tile([P, ko, P], mmdt, name="w_sb")
                nc.sync.dma_start(w0[:], wT[0])
                for k in range(ko):
                    nc.sync.dma_start(x_sb[:, k], xT[sb, :, k])

                # xaT[r, t] = sigm * (A @ x^T); sigma folds into xa on the
                # PSUM copy-out, so the epilogue matmuls use raw B^T slices.
                # xa is then replicated at partition offsets 32/64/96 to
                # match the packed epilogue row groups.
                xa_sb = lp.tile([P, tt_free], mmdt, tag=f"xa{sb}")
                xa_ps = pmain.tile([r, tt_free], f32,
                                   name="xa_ps", tag="ps")
                for k in range(ko):
                    nc.tensor.matmul(
                        xa_ps[:], a_sb[:, k, :], x_sb[:, k],
                        start=(k == 0), stop=(k == ko - 1))
                nc.vector.tensor_tensor(
                    xa_sb[:r], xa_ps[:],
                    sig_sb[:, 2:3].to_broadcast((r, tt_free)),
                    mybir.AluOpType.mult)
                for j in range(1, 4):
                    nc.sync.dma_start(
                        xa_sb[32 * j:32 * j + r, :], xa_sb[:r, :])

                # main loop in quads of o-tiles: W^T chunk stationary, x^T
                # moving; the 4 lora epilogues of a quad are packed into
                # disjoint PE row groups so they run concurrently
                for q in range(ot // 4):
                    ws, pss = [], []
                    for j in range(4):
                        if q == 0 and j == 0:
                            ws.append(w0)  # prefetched before the x chunks
                        else:
                            w_sb = wp.tile([P, ko, P], mmdt, name="w_sb")
                            nc.sync.dma_start(w_sb[:], wT[4 * q + j])
                            ws.append(w_sb)
                        pss.append(pmain.tile([P, tt_free], f32,
                                              name=f"ps{j}", tag="ps"))
                    for j in range(4):
                        for k in range(ko):
                            nc.tensor.matmul(
                                pss[j][:], ws[j][:, k, :], x_sb[:, k],
                                start=(k == 0), stop=False)
                    # adjacent packed epilogues -> concurrent row groups
                    for j in range(4):
                        nc.tensor.matmul(
                            pss[j][:],
                            b_all[32 * j:32 * j + r, ts(4 * q + j, P)],
                            xa_sb[32 * j:32 * j + r, :],
                            start=False, stop=True,
                            tile_position=(32 * j, 0))
                    for j in range(4):
                        o_sb = op.tile([P, tt_free], f32)
                        nc.vector.tensor_copy(out=o_sb[:], in_=pss[j][:])
                        nc.sync.dma_start(
                            outT[ts(4 * q + j, P), ts(sb, tt_free)], o_sb[:])
    return nc


def make_in_maps(x, weight, lora_A, lora_B, lora_sigma, mm_dtype=MM_DTYPE):
    """Host-side layout prep (transpose/tile/slice/dtype-cast only)."""
    npdt = mybir.dt.np(getattr(mybir.dt, mm_dtype))
    xf = np.asarray(x, dtype=np.float32).astype(npdt).reshape(T, D)
    wT_t = np.ascontiguousarray(
        np.asarray(weight, dtype=np.float32).astype(npdt).T
        .reshape(KO, P, OT, P).transpose(2, 1, 0, 3))            # [ot, ki, ko, o128]
    aT_t = np.ascontiguousarray(
        np.asarray(lora_A, dtype=np.float32).astype(npdt).T
        .reshape(KO, P, R).transpose(1, 0, 2))                   # [ki, ko, r]
    bT = np.ascontiguousarray(
        np.asarray(lora_B, dtype=np.float32).astype(npdt).T)     # [r, o]
    sg = np.ascontiguousarray(lora_sigma, dtype=np.float32).reshape(R, 1)

    in_maps = []
    for c in range(N_CORES):
        # [sb, ki, ko, t]
        xTc = np.ascontiguousarray(
            xf[c * TC:(c + 1) * TC]
            .reshape(SB_N, TT_FREE, KO, P).transpose(0, 3, 2, 1))
        in_maps.append(
            {"xT": xTc, "wT": wT_t, "aT": aT_t, "bT": bT, "sig": sg})
    return in_maps


def _gather(res):
    out = np.empty((T, D), dtype=np.float32)
    for c in range(N_CORES):
        out[c * TC:(c + 1) * TC] = res.results[c]["out"].T
    return out.reshape(B, S, D)


def kernel(x, weight, lora_A, lora_B, lora_sigma, _trace=False, _repeat=1):
    in_maps = make_in_maps(x, weight, lora_A, lora_B, lora_sigma)
    nc = build_nc()
    nc.finalize()

    def run_once():
        return run_bass_kernel_spmd(
            nc, in_maps, core_ids=list(range(N_CORES)), trace=_trace)

    res = None
    for attempt in range(3):
        try:
            res = run_once()
            out = _gather(res)
            if not np.isnan(out).any():
                break
        except Exception:
            if attempt == 2:
                raise
    extra = [run_once() for _ in range(_repeat - 1)]
    out = _gather(res)
    if _trace:
        return out, [res, *extra]
    return out
